# revision 11
# baseline (speedup 1.0000x reference)
"""Trainium2 Bass kernel for nn_MultiHeadAttention_2250562863251.

Key algebraic insight: the reference einsum 'mbhi,nbhj->mnbh' contracts i and j
independently, so scores[m,n,b,h] = (sum_i q[m,b,h,i]) * (sum_j k[n,b,h,j]) --
a rank-1 outer product of per-head row-sums. Full Q/K projections are never
needed; only queries @ (per-head-summed Wq) [E,16], computed on host (tiny).

Sharding: 8 cores = 2 (batch) x 4 (head-groups of 4 heads). SPMD program via
run_bass_kernel_spmd; host shards inputs / gathers + reduces outputs.

v3 architecture (v1: DVE+ACT bound at ~197us; v2 showed DVE stt has no 2x
uop -- 1x at any dtype -- so score building must leave the DVE entirely):
  - scores built BY THE PE as K=2 rank-2 matmuls into PSUM:
    sc[n,m] = c_n*qs_m + 1*beta_m with lhsT=[c;1] (fp16, per head+chunk) and
    rhs=[qs;beta] (fp16, per head). Kills the DVE stt AND all 6MB of
    broadcast tiles (qs/beta/beta+tri), fixing the DMA-bound startup.
  - padding mask folded into V (host zeroes masked valuesT columns; tiny
    ones-mask DMA zeroes denominator rows) so exp needs no bias.
  - exp fused per (n-chunk, head-pair) group [128,1024] straight from PSUM.
  - causal triangle: post-exp DVE tensor_mul with a constant [128,128] 0/1
    mask on the single triangular 128-col block of each diagonal chunk.
  - 2-head passes keep PSUM inside 8 banks: sc groups (2 banks x 2 bufs) +
    2 pool banks + 2 out-proj banks.
  - softmax divide from SBUF fp16 at 2x after an ACT evacuation copy whose
    denominator row rides along; reciprocal via [128,8] partition-spread.
  - epilogues software-pipelined one pass behind so DMA round-trips never
    stall an engine stream.
"""
import sys

for _p in ("/opt/trn_rl_repo", "/root/.axon_site/_ro/trn_rl_repo"):
    if _p not in sys.path:
        sys.path.append(_p)

import numpy as np
import ml_dtypes

import concourse.bass as bass
import concourse.mybir as mybir
import concourse.tile as tile
from concourse import bacc
from concourse.bass_utils import run_bass_kernel_spmd

# Problem shapes (hardcoded per contract)
M = 2048   # query positions
N = 2048   # key positions
B = 2
E = 1024
H = 16
DH = 64        # head dim
HL = 4         # heads per core
KL = HL * DH   # 256 local pooled dims
NEG = -1000.0
P = 128
NK = N // P    # 16 n-chunks
T = 4          # m-tiles of 512
MT = 512
NCORES = 8

f32 = mybir.dt.float32
f16 = mybir.dt.float16
bf16 = mybir.dt.bfloat16

_CACHE = {}


def _build_program():
    if "nc" in _CACHE:
        return _CACHE["nc"]
    nc = bacc.Bacc("TRN2", target_bir_lowering=False, debug=False,
                   num_devices=NCORES)

    vt_d = nc.declare_dram_parameter("vt", [P, 4, (E // P) * MT], bf16, isOutput=False)
    wvlt_d = nc.declare_dram_parameter("wvlt", [P, (E // P) * KL], bf16, isOutput=False)
    wolt_d = nc.declare_dram_parameter("wolt", [P, (KL // P) * E], f16, isOutput=False)
    ck_d = nc.declare_dram_parameter("ck", [2, HL * NK * P], f16, isOutput=False)
    qb_d = nc.declare_dram_parameter("qb", [2, HL * M], f16, isOutput=False)
    ones_d = nc.declare_dram_parameter("onesm", [P, NK * HL], bf16, isOutput=False)
    tri_d = nc.declare_dram_parameter("tri01", [P, P], f32, isOutput=False)
    # blocked output: [ob, t, 128, 512] -> host reassembles to [E, M]
    outp_d = nc.declare_dram_parameter("outp", [E // P, T, P, MT], f32,
                                       isOutput=True)

    with tile.TileContext(nc) as tc:
        with (
            tc.tile_pool(name="const", bufs=1) as const,
            tc.tile_pool(name="vstream", bufs=2) as vstream,
            tc.tile_pool(name="etpool", bufs=3) as etpool,
            tc.tile_pool(name="rspool", bufs=1) as rspool,
            tc.tile_pool(name="ptn", bufs=2) as ptn,
            tc.tile_pool(name="small", bufs=2) as small,
            tc.tile_pool(name="opool", bufs=3) as opool,
            tc.tile_pool(name="dpool", bufs=2, space="DRAM") as dpool,
            tc.tile_pool(name="ps_sc", bufs=2, space="PSUM") as ps_sc,
            tc.tile_pool(name="ps_v", bufs=2, space="PSUM") as ps_v,
            tc.tile_pool(name="ps_pool", bufs=1, space="PSUM") as ps_pool,
        ):
            # ---- resident constants (ordered so PE starts ASAP) ----
            wvlt_sb = const.tile([P, E // P, KL], bf16)
            nc.sync.dma_start(wvlt_sb[:], wvlt_d.rearrange("p (ek d) -> p ek d", ek=E // P))
            ck_sb = const.tile([2, HL, NK * P], f16)
            nc.sync.dma_start(ck_sb[:], ck_d.rearrange("p (h x) -> p h x", h=HL))
            qb_sb = const.tile([2, HL, M], f16)
            nc.sync.dma_start(qb_sb[:], qb_d.rearrange("p (h m) -> p h m", h=HL))
            tri_sb = const.tile([P, P], f32)
            nc.sync.dma_start(tri_sb[:], tri_d[:, :])

            # v_sb[:, k, h*65 : h*65+64] = v for head h, chunk k; col 64 =
            # 1.0 where key is live, 0.0 where padded (denominator mask).
            v_sb = const.tile([P, NK, HL * (DH + 1)], bf16)
            nc.sync.dma_start(
                v_sb.rearrange("p k (h x) -> p k h x", x=DH + 1)[:, :, :, DH:DH + 1],
                ones_d.rearrange("p (k h x) -> p k h x", k=NK, x=1))

            # ---- stage 1: v projection (vt streamed per n-quarter) ----
            # quarters run DESCENDING: stage-2's k-loop consumes v_sb[15] first
            for q in range(3, -1, -1):
                vt_sb = vstream.tile([P, E // P, MT], bf16, tag="vt")
                nc.sync.dma_start(
                    vt_sb[:], vt_d[:, q].rearrange("p (ek n) -> p ek n", ek=E // P))
                for nk_r in range(3, -1, -1):
                    k = q * 4 + nk_r
                    vpsf = ps_v.tile([P, MT], f32, tag="ops", name=f"vps{k}")
                    vps = vpsf[:, 0:KL]
                    for ek in range(E // P):
                        nc.tensor.matmul(
                            vps[:],
                            vt_sb[:, ek, nk_r * P:(nk_r + 1) * P],
                            wvlt_sb[:, ek, :],
                            start=(ek == 0),
                            stop=(ek == E // P - 1),
                        )
                    nc.vector.tensor_copy(
                        out=v_sb[:, k].rearrange("p (h x) -> p h x", x=DH + 1)[:, :, 0:DH],
                        in_=vps.rearrange("p (h x) -> p h x", x=DH),
                    )

            wolt_sb = const.tile([P, KL // P, E], f16)
            nc.sync.dma_start(wolt_sb[:], wolt_d.rearrange("p (kb o) -> p kb o", kb=KL // P))

            # ---- stage 2: 8 passes pi = 2t + hp, heads {2hp, 2hp+1} ----
            pool_sb_l = [None] * 16          # evacuated pools by (t, h)
            pdiv_l = [None] * 16             # divided pools by (t, h)
            rdall_l = [None] * 8             # denom rows by pass
            rsg_l = [None] * 8
            NPASS = 2 * T

            def emit_pass(pi):
                t, hp = pi // 2, pi % 2
                heads = (2 * hp, 2 * hp + 1)
                ts = t * MT
                pools = {}
                for h in heads:
                    pools[h] = ps_pool.tile([DH + 1, MT], f32, tag=f"pool{h % 2}",
                                            name=f"pool_{pi}_{h}")
                for k in range(NK - 1, 4 * t - 1, -1):
                    pos = k - 4 * t
                    W = MT if pos >= 4 else (pos + 1) * P
                    sc = ps_sc.tile([P, 2 * MT], f32, tag="sc", name=f"sc{pi}_{k}")
                    for j, h in enumerate(heads):
                        nc.tensor.matmul(
                            sc[:, j * MT:j * MT + W],
                            ck_sb[:, h, k * P:(k + 1) * P],
                            qb_sb[:, h, ts:ts + W],
                            start=True, stop=True,
                        )
                    if pos < 4:
                        # pre-exp causal mask: -30000 on the upper triangle
                        # of the last 128-col block (sc bounded by ~2200,
                        # so masked entries exp to exactly 0)
                        lw = pos * P
                        for j in range(2):
                            nc.vector.tensor_add(
                                out=sc[:, j * MT + lw:j * MT + lw + P],
                                in0=sc[:, j * MT + lw:j * MT + lw + P],
                                in1=tri_sb[:],
                            )
                    et = etpool.tile([P, 2 * MT], bf16, tag="et", name=f"et{pi}_{k}")
                    if W == MT:
                        nc.scalar.activation(et[:], sc[:],
                                             mybir.ActivationFunctionType.Exp)
                    else:
                        for j in range(2):
                            nc.scalar.activation(
                                et[:, j * MT:j * MT + W], sc[:, j * MT:j * MT + W],
                                mybir.ActivationFunctionType.Exp)
                    for j, h in enumerate(heads):
                        nc.tensor.matmul(
                            pools[h][:, 0:W],
                            v_sb[:, k, h * (DH + 1):(h + 1) * (DH + 1)],
                            et[:, j * MT:j * MT + W],
                            start=(k == NK - 1 or (t == 3 and pos == 3)),
                            stop=(pos == 0),
                        )
                # evacuate pools (ACT, fp16; denominator row rides along)
                rdall_l[pi] = dpool.tile([2, MT], f16, tag=f"rd{pi % 2}",
                                         name=f"rdall{pi}")
                for j, h in enumerate(heads):
                    pool_sb = rspool.tile([DH + 1, MT], f16,
                                          tag=f"pool_sb{(4 * t + h) % 8}",
                                          name=f"pool_sb_{pi}_{h}")
                    nc.scalar.copy(pool_sb[:], pools[h][:])
                    pool_sb_l[4 * t + h] = pool_sb
                    nc.sync.dma_start(rdall_l[pi][j:j + 1, :], pool_sb[DH:DH + 1, :])
                rsg = small.tile([P, 2 * MT // P], f16, tag=f"rsg{pi % 2}",
                                 name=f"rsg{pi}")
                nc.sync.dma_start(
                    rsg[:], rdall_l[pi].rearrange("a (b x) -> (a b) x", x=2 * MT // P))
                rsg_l[pi] = rsg

            def emit_divides(pi):
                # reciprocal + divide for pass pi (issued one pass later)
                t, hp = pi // 2, pi % 2
                heads = (2 * hp, 2 * hp + 1)
                rsgr = small.tile([P, 2 * MT // P], f16, tag=f"rsgr{pi % 2}",
                                  name=f"rsgr{pi}")
                with nc.allow_low_precision(reason="per-(m,h) softmax scale"):
                    nc.vector.reciprocal(out=rsgr[:], in_=rsg_l[pi][:])
                rdall2 = dpool.tile([2, MT], f16, tag=f"rd2{pi % 2}",
                                    name=f"rdall2{pi}")
                nc.sync.dma_start(
                    rdall2.rearrange("a (b x) -> (a b) x", x=2 * MT // P), rsgr[:])
                for j, h in enumerate(heads):
                    rsb = small.tile([DH, MT], f16, tag=f"rsb{h % 2}",
                                     name=f"rsb{pi}_{h}")
                    nc.sync.dma_start(
                        rsb[:], rdall2[j][None, :].to_broadcast([DH, MT]))
                    pdiv = ptn.tile([DH, MT], f16, tag=f"pdiv{h}",
                                    name=f"pdiv{pi}_{h}")
                    nc.vector.tensor_mul(
                        out=pdiv[:],
                        in0=pool_sb_l[4 * t + h][0:DH, :],
                        in1=rsb[:],
                    )
                    pdiv_l[4 * t + h] = pdiv

            def emit_outproj(t):
                # pair heads into 128 partitions for K=128 out-proj matmuls
                pTn2 = ptn.tile([P, KL // P, MT], f16, tag="ptn2",
                                name=f"ptn2_{t}")
                for kb in range(KL // P):
                    nc.sync.dma_start(pTn2[0:DH, kb], pdiv_l[4 * t + 2 * kb][:])
                    nc.sync.dma_start(pTn2[DH:P, kb], pdiv_l[4 * t + 2 * kb + 1][:])
                for ob in range(E // P):
                    ops = ps_v.tile([P, MT], f32, tag="ops", name=f"ops{t}_{ob}")
                    for kb in range(KL // P):
                        nc.tensor.matmul(
                            ops[:],
                            wolt_sb[:, kb, ob * P:(ob + 1) * P],
                            pTn2[:, kb, :],
                            start=(kb == 0),
                            stop=(kb == KL // P - 1),
                        )
                    osb = opool.tile([P, MT], f32, tag="osb", name=f"osb{t}_{ob}")
                    nc.vector.tensor_copy(out=osb[:], in_=ops[:])
                    nc.sync.dma_start(outp_d[ob, t], osb[:])

            for pi in range(NPASS):
                emit_pass(pi)
                if pi >= 1:
                    emit_divides(pi - 1)
                if pi >= 3 and pi % 2 == 1:
                    emit_outproj(pi // 2 - 1)
            emit_divides(NPASS - 1)
            emit_outproj(T - 1)

    nc.compile()
    _CACHE["nc"] = nc
    return nc


def _host_prep(queries, keys, values, Wq, bq, Wk, bk, Wv, bv, Wo, bo, in_mask):
    """Host-side prep. Returns (in_maps, fixup, extras)."""
    qs = np.einsum("mbe,he->mbh", queries, Wq.reshape(H, DH, E).sum(1),
                   dtype=np.float32) + bq.reshape(H, DH).sum(1)
    ks = np.einsum("nbe,he->nbh", keys, Wk.reshape(H, DH, E).sum(1),
                   dtype=np.float32) + bk.reshape(H, DH).sum(1)
    # device multiplies fp16-rounded qs and c; compute beta from the same
    qs16 = qs.astype(np.float16)
    qsf = qs16.astype(np.float32)

    mask3 = in_mask[:, :, None]
    cp = np.where(mask3, 0.0, ks).astype(np.float16)          # [n, b, H] fp16
    cpf = cp.astype(np.float32)

    cmax = np.where(mask3, -np.inf, cpf)
    cmax = np.maximum.accumulate(cmax[::-1], axis=0)[::-1]    # suffix max, n>=m
    cmin = np.where(mask3, np.inf, cpf)
    cmin = np.minimum.accumulate(cmin[::-1], axis=0)[::-1]
    nonempty = np.maximum.accumulate((~in_mask)[::-1], axis=0)[::-1]  # [n, b]

    with np.errstate(invalid="ignore"):
        A = np.where(qsf >= 0, qsf * cmax, qsf * cmin)        # [m, b, H]
    A = np.where(nonempty[:, :, None], A, -np.inf)
    fixup_rows = np.any(~(A > -70.0), axis=2)                 # [m, b] (nan-safe)
    beta = np.where(np.isfinite(A), -A, 1e4)
    beta = np.where(fixup_rows[:, :, None], -1e4, beta)
    beta = beta.astype(np.float32)

    in_maps = []
    def pmajor(a, p=P):
        """[X*p, Y] -> [p, X*Y]: partition-major packing for 1-run-per-
        partition DMA loads matching 'p (x y) -> p x y' device views."""
        X = a.shape[0] // p
        return np.ascontiguousarray(
            a.reshape(X, p, a.shape[1]).transpose(1, 0, 2).reshape(p, -1))

    def pack_vt(vT):
        # [E, N] -> [P, 4, (E//P)*MT]: quarter-major, then ek-major
        a = vT.reshape(E // P, P, 4, MT)          # [ek, p, q, mt]
        return np.ascontiguousarray(
            a.transpose(1, 2, 0, 3).reshape(P, 4, (E // P) * MT))

    # zero masked key columns of v^T: their pooled contribution must vanish
    vt_by_b = []
    for bi in range(B):
        vT = values[:, bi, :].T.copy()
        vT[:, in_mask[:, bi]] = 0.0
        vt_by_b.append(pack_vt(vT.astype(ml_dtypes.bfloat16)))

    # ones-column mask [P, NK, HL]: 1.0 for live keys, 0.0 for padded
    live = (~in_mask).astype(np.float32)                      # [n, b]
    onesm_by_b = [
        np.ascontiguousarray(np.broadcast_to(
            live[:, bi].reshape(NK, P, 1).transpose(1, 0, 2), (P, NK, HL))
        ).reshape(P, NK * HL).astype(ml_dtypes.bfloat16)
        for bi in range(B)]

    # causal mask add: -30000 where p < j (key idx within chunk < query)
    tri01 = np.where(np.arange(P)[:, None] < np.arange(P)[None, :], -30000.0,
                     0.0).astype(np.float32)

    for c in range(NCORES):
        bi, hg = c // 4, c % 4
        lh = slice(hg * HL, (hg + 1) * HL)
        ds = slice(hg * KL, (hg + 1) * KL)
        # ck [2, HL*N]: row 0 = c values, row 1 = ones
        ckh = np.empty((2, HL, N), np.float16)
        ckh[0] = cp[:, bi, lh].T
        ckh[1] = 1.0
        # qb [2, HL*M]: row 0 = qs, row 1 = beta
        qbh = np.empty((2, HL, M), np.float16)
        qbh[0] = qs16[:, bi, lh].T
        qbh[1] = beta[:, bi, lh].T.astype(np.float16)
        in_maps.append({
            "vt": vt_by_b[bi],
            "wvlt": pmajor(Wv[ds, :].T.astype(ml_dtypes.bfloat16)),
            "wolt": pmajor(Wo[:, ds].T.astype(np.float16)),
            "ck": np.ascontiguousarray(ckh.reshape(2, HL * N)),
            "qb": np.ascontiguousarray(qbh.reshape(2, HL * M)),
            "onesm": onesm_by_b[bi],
            "tri01": tri01,
        })
    return in_maps, fixup_rows, (qsf, ks)


def _fixup_row(out, m, bi, qs, ks, values, Wv, bv, Wo, bo, in_mask):
    """Exact numpy recompute of one output row (degenerate / extreme rows)."""
    pot = qs[m, bi, :][None, :] * ks[:, bi, :]                # [n, H]
    pot = np.where(in_mask[:, bi][:, None], NEG, pot)
    causal = np.arange(N) < m                                 # mask n < m
    pot = np.where(causal[:, None], NEG, pot)
    pot = pot - pot.max(axis=0, keepdims=True)
    w = np.exp(pot)
    w = w / w.sum(axis=0, keepdims=True)                      # [n, H]
    v = (values[:, bi, :] @ Wv.T + bv).reshape(N, H, DH)
    pooled = np.einsum("nh,nhd->hd", w, v).reshape(E)
    out[m, bi, :] = pooled @ Wo.T + bo


def kernel(queries, keys, values, Wq, bq, Wk, bk, Wv, bv, Wo, bo, in_mask,
           _trace=False):
    args = (queries, keys, values, Wq, bq, Wk, bk, Wv, bv, Wo, bo)
    args = tuple(np.asarray(a, np.float32) for a in args)
    in_mask = np.asarray(in_mask, bool)
    (queries, keys, values, Wq, bq, Wk, bk, Wv, bv, Wo, bo) = args

    nc = _build_program()
    in_maps, fixup_rows, (qs, ks) = _host_prep(
        queries, keys, values, Wq, bq, Wk, bk, Wv, bv, Wo, bo, in_mask)

    res = run_bass_kernel_spmd(nc, in_maps, list(range(NCORES)), trace=_trace)
    results = res.results

    out = np.zeros((M, B, E), np.float32)
    for c in range(NCORES):
        bi = c // 4
        blk = np.asarray(results[c]["outp"], np.float32)   # [8, 4, 128, 512]
        outT = blk.transpose(0, 2, 1, 3).reshape(E, M)
        out[:, bi, :] += outT.T
    out += (bo + bv @ Wo.T)[None, None, :]

    for m, bi in zip(*np.nonzero(fixup_rows)):
        _fixup_row(out, m, bi, qs, ks, values, Wv, bv, Wo, bo, in_mask)

    if _trace:
        return out, res
    return out
